# revision 12
# baseline (speedup 1.0000x reference)
"""CapsNet dynamic-routing layer on 8 Trainium2 NeuronCores.

Sharding: tensor-parallel over num_caps_j (J=32 -> 4 per core). Every
(batch, j) pair's routing is independent, so there are no collectives:
each core computes u_hat[:, :, j_shard, :] plus 3 routing iterations and
returns v_J[:, j_shard, :].

Per-core dataflow:
  - einsum u_hat[b,i,jv] = sum_d W[i,d,jv] * u[b,i,d] as 1024 PE matmuls
    (lhsT = x chunk [d,b], rhs = W chunk [d,jv]), fp16 operands, fp32 PSUM.
  - s0 = sum_i u_hat accumulated in a dedicated PSUM bank by duplicate
    matmuls (PE has slack; saves a full DVE pass).
  - u_hat stored SBUF-resident as fp16 [b, j, i, v] (16.8 MB).
  - routing iterations: t-pass (u_hat . v) and s-pass (sum_i c*u_hat) on
    the vector engine with fold trees; softmax exp on scalar engine.
"""

import sys

if "/opt/trn_rl_repo" not in sys.path:
    sys.path.insert(0, "/opt/trn_rl_repo")

import numpy as np

B, I, D, J, V = 128, 512, 256, 32, 32
NCORES = 8
JL = J // NCORES          # 4 j's per core
JV = JL * V               # 128
DP = 128                  # contraction chunk (partitions)
EPS = 1e-9
IBLK = 32                 # i-block per DMA tile
NCH = 4                   # routing i-chunks
CHUNK = I // NCH          # 128 i's per routing chunk

_cache = {}


def _build_program():
    import concourse.bass as bass
    import concourse.tile as tile
    from concourse import bacc, mybir

    f16 = mybir.dt.float16
    f32 = mybir.dt.float32

    nc = bacc.Bacc("TRN2", target_bir_lowering=False, debug=False,
                   num_devices=NCORES)

    xa = nc.dram_tensor("xa", [DP, I, B], f16, kind="ExternalInput")
    xb = nc.dram_tensor("xb", [DP, I, B], f16, kind="ExternalInput")
    wa = nc.dram_tensor("wa", [DP, I, JV], f16, kind="ExternalInput")
    wb = nc.dram_tensor("wb", [DP, I, JV], f16, kind="ExternalInput")
    v2d = nc.dram_tensor("v2", [B, JV], f32, kind="ExternalOutput")

    with tile.TileContext(nc) as tc:
        with (
            tc.tile_pool(name="uhat", bufs=1) as upool,
            tc.tile_pool(name="ps0", bufs=1, space="PSUM") as ps0pool,
        ):
            U = upool.tile([B, JL, I, V], f16)      # 128 KB/partition
            ps0 = ps0pool.tile([B, JV], f32)

            # ---------------- Phase 1: einsum ----------------
            with (
                tc.tile_pool(name="xw", bufs=2) as xwpool,
                tc.tile_pool(name="psum", bufs=4, space="PSUM") as pspool,
            ):
                nblk = I // IBLK
                last_i = I - 1
                for blk in range(nblk):
                    i0 = blk * IBLK
                    xa_t = xwpool.tile([DP, IBLK, B], f16, tag="xa")
                    nc.sync.dma_start(xa_t[:], xa.ap()[:, i0:i0 + IBLK, :])
                    xb_t = xwpool.tile([DP, IBLK, B], f16, tag="xb")
                    nc.sync.dma_start(xb_t[:], xb.ap()[:, i0:i0 + IBLK, :])
                    wa_t = xwpool.tile([DP, IBLK, JV], f16, tag="wa")
                    nc.sync.dma_start(wa_t[:], wa.ap()[:, i0:i0 + IBLK, :])
                    wb_t = xwpool.tile([DP, IBLK, JV], f16, tag="wb")
                    nc.sync.dma_start(wb_t[:], wb.ap()[:, i0:i0 + IBLK, :])

                    for g in range(IBLK // 4):
                        ps = pspool.tile([B, 4, JV], f32)
                        for k in range(4):
                            il = g * 4 + k
                            i_abs = i0 + il
                            nc.tensor.matmul(
                                ps[:, k, :], xa_t[:, il, :], wa_t[:, il, :],
                                start=True, stop=False)
                            nc.tensor.matmul(
                                ps[:, k, :], xb_t[:, il, :], wb_t[:, il, :],
                                start=False, stop=True)
                            # s0 accumulation (sum over all i)
                            nc.tensor.matmul(
                                ps0[:], xa_t[:, il, :], wa_t[:, il, :],
                                start=(i_abs == 0), stop=False,
                                skip_group_check=True)
                            nc.tensor.matmul(
                                ps0[:], xb_t[:, il, :], wb_t[:, il, :],
                                start=False, stop=(i_abs == last_i),
                                skip_group_check=True)
                        # PSUM [b, i4, (j v)] -> SBUF U[b, j, i0+g*4:+4, v]
                        ia = i0 + g * 4
                        src = ps.rearrange("p i (j v) -> p j i v", j=JL)
                        dst = U[:, :, ia:ia + 4, :]
                        if g % 2 == 0:
                            nc.scalar.copy(dst, src)
                        else:
                            nc.vector.tensor_copy(dst, src)

            # ---------------- Phase 2: routing ----------------
            mybir_ = mybir
            from contextlib import ExitStack
            _stack = ExitStack()
            rpool = _stack.enter_context(tc.tile_pool(name="rout", bufs=1))
            s_acc = rpool.tile([B, JL, V], f32)
            w16 = rpool.tile([B, JL, V], f16)
            bij = rpool.tile([B, JL, I], f32)
            e16 = rpool.tile([B, JL, I], f16)
            c16 = rpool.tile([B, JL, I], f16)
            Ssum = rpool.tile([B, JL], f32)
            Srec = rpool.tile([B, JL], f32)
            cfac = rpool.tile([B, JL], f16)
            n2 = rpool.tile([B, JL], f32)
            sq = rpool.tile([B, JL, V], f32)
            d1 = rpool.tile([B, JL], f32)
            r1 = rpool.tile([B, JL], f32)
            rt = rpool.tile([B, JL], f32)
            r2 = rpool.tile([B, JL], f32)
            fac = rpool.tile([B, JL], f32)
            vout = rpool.tile([B, JL, V], f32)
            eps_t = rpool.tile([B, 1], f32)
            nc.gpsimd.memset(eps_t[:], EPS)

            def squash(s_ap, v_ap):
                # v = s * n2/(1+n2)/sqrt(n2+EPS), per (b, j) over v-axis
                nc.vector.tensor_mul(sq[:], s_ap, s_ap)
                nc.vector.reduce_sum(n2[:], sq[:], axis=mybir_.AxisListType.X)
                nc.scalar.add(d1[:], n2[:], 1.0)
                nc.vector.reciprocal(r1[:], d1[:])
                nc.scalar.activation(rt[:], n2[:],
                                     mybir_.ActivationFunctionType.Sqrt,
                                     bias=eps_t[:])
                nc.vector.reciprocal(r2[:], rt[:])
                nc.vector.tensor_mul(fac[:], n2[:], r1[:])
                nc.vector.tensor_mul(fac[:], fac[:], r2[:])
                fb = fac[:].unsqueeze(2).broadcast_to([B, JL, V])
                nc.vector.tensor_tensor(v_ap, s_ap, fb,
                                        op=mybir_.AluOpType.mult)

            # s0 from PSUM
            nc.vector.tensor_copy(
                s_acc[:], ps0.rearrange("p (j v) -> p j v", j=JL))
            squash(s_acc[:], vout[:])
            nc.vector.tensor_copy(w16[:], vout[:])  # w = v0 (fp16)

            ppool = _stack.enter_context(tc.tile_pool(name="prod", bufs=2))
            if True:
                for r in (1, 2):
                    # ---- t-pass: tdel[b,j,i] = sum_v U*w16 ----
                    for j in range(JL):
                        for h in range(NCH):
                            isl = slice(h * CHUNK, (h + 1) * CHUNK)
                            prod = ppool.tile([B, CHUNK, V], f16, tag="prod")
                            wb_ = (w16[:, j, :].unsqueeze(1)
                                   .broadcast_to([B, CHUNK, V]))
                            nc.vector.tensor_tensor(
                                prod[:], U[:, j, isl, :], wb_,
                                op=mybir_.AluOpType.mult)
                            # b_r = U . w_r exactly (w is cumulative sum of
                            # v's and b starts at 0), so overwrite, not add.
                            nc.vector.reduce_sum(
                                bij[:, j, isl], prod[:],
                                axis=mybir_.AxisListType.X)

                    # ---- softmax over i (no max-sub; |b| small) ----
                    nc.scalar.activation(e16[:], bij[:],
                                         mybir_.ActivationFunctionType.Exp)
                    nc.vector.reduce_sum(Ssum[:], e16[:],
                                         axis=mybir_.AxisListType.X)
                    nc.vector.reciprocal(Srec[:], Ssum[:])
                    nc.scalar.mul(cfac[:], Srec[:], float(I))
                    cb = cfac[:].unsqueeze(2).broadcast_to([B, JL, I])
                    nc.vector.tensor_tensor(c16[:], e16[:], cb,
                                            op=mybir_.AluOpType.mult)

                    # ---- s-pass: s[b,j,v] = sum_i c16*U ----
                    for j in range(JL):
                        for h in range(NCH):
                            isl = slice(h * CHUNK, (h + 1) * CHUNK)
                            prod = ppool.tile([B, CHUNK, V], f16, tag="prod")
                            cb_ = (c16[:, j, isl].unsqueeze(2)
                                   .broadcast_to([B, CHUNK, V]))
                            nc.vector.tensor_tensor(
                                prod[:], U[:, j, isl, :], cb_,
                                op=mybir_.AluOpType.mult)
                            # fold tree over i: HALF -> 1
                            cur = prod
                            n = CHUNK
                            while n > 1:
                                nh = n // 2
                                nxt = ppool.tile([B, nh, V], f16,
                                                 tag=f"fold{nh}")
                                nc.vector.tensor_add(
                                    nxt[:], cur[:, 0:nh, :], cur[:, nh:n, :])
                                cur = nxt
                                n = nh
                            if h == 0:
                                nc.vector.tensor_copy(s_acc[:, j, :],
                                                      cur[:, 0, :])
                            else:
                                nc.vector.tensor_add(
                                    s_acc[:, j, :], s_acc[:, j, :],
                                    cur[:, 0, :])
                    squash(s_acc[:], vout[:])
                    if r == 1:
                        # w += v1
                        nc.vector.tensor_add(
                            w16[:], w16[:],
                            vout[:])
                    else:
                        nc.sync.dma_start(
                            v2d.ap(),
                            vout[:].rearrange("p j v -> p (j v)"))
            _stack.close()

    nc.compile()
    return nc


def _get_program():
    if "nc" not in _cache:
        _cache["nc"] = _build_program()
    return _cache["nc"]


def _prep_inputs(x, W):
    """Host-side shard + transpose + fp16 cast."""
    u = np.ascontiguousarray(x[..., 0])                   # [B, I, D] f32
    xt = np.ascontiguousarray(u.transpose(2, 1, 0)).astype(np.float16)
    xa_np = np.ascontiguousarray(xt[:DP])                 # [128, I, B]
    xb_np = np.ascontiguousarray(xt[DP:])
    W0 = W[0]                                             # [I, J, D, V]
    in_maps = []
    for c in range(NCORES):
        Wc = W0[:, c * JL:(c + 1) * JL]                   # [I, JL, D, V]
        Wt = Wc.transpose(2, 0, 1, 3)                     # [D, I, JL, V]
        Wt = Wt.reshape(D, I, JV).astype(np.float16)
        in_maps.append({
            "xa": xa_np,
            "xb": xb_np,
            "wa": np.ascontiguousarray(Wt[:DP]),
            "wb": np.ascontiguousarray(Wt[DP:]),
        })
    return in_maps


def run_cores(x, W, trace=False):
    from concourse import bass_utils
    nc = _get_program()
    in_maps = _prep_inputs(x, W)
    res = bass_utils.run_bass_kernel_spmd(
        nc, in_maps, core_ids=list(range(NCORES)), trace=trace)
    return res


def kernel(x, W):
    x = np.asarray(x)
    W = np.asarray(W)
    res = run_cores(x, W, trace=False)
    out = np.empty((B, J, V, 1), dtype=np.float32)
    for c in range(NCORES):
        vc = res.results[c]["v2"].reshape(B, JL, V)
        out[:, c * JL:(c + 1) * JL, :, 0] = vc
    return out
